# revision 16
# baseline (speedup 1.0000x reference)
"""GAT attention layer (EEGGraphAttentionLayer) for Trainium2, 8 NeuronCores.

reference math:
    Wh = h @ w                         # (8192, 64)
    e  = leaky_relu((Wh@a_src) + (Wh@a_dst).T, slope=0.2)   # (8192, 8192)
    att = where(adj > 0, e, -1e12)
    out = softmax(att, axis=1)

Sharding: rows of adj/out across 8 cores (1024 rows each); row softmax is
core-local. Each core recomputes the column-score vector s2 = h @ (w@a_dst)
(an N-vector) from the full h instead of communicating. h and w are passed
host-transposed and h/adj precision-reduced (bf16 / fp8e5 -- pure dtype
compression, no host arithmetic); the output is stored bf16 and upcast to
fp32 on the host (exact upcast).

Per-core device pipeline (row tile = [128, 8192] bf16, C = 2^-40 exact):
    wa12 = wT.T @ [a_src|a_dst]                      (PE)
    s1c[:, t] = C * (hsT_tile.T @ wa1)               (PE)   per-row bias
    BC2[p, j] = C * s2[j] via (C*wa2 bcast).T @ hT   (PE)   16 x 512 chunks
    s2maxc    = rowmax(BC2)                          (DVE reduce, one-time)
    Mc[:, t]  = lrelu(s1c + s2maxc)  == C * rowmax(e)  (exact: lrelu monotone)
    r = lrelu(min(BC2 + s1c, adj)) - Mc              (DVE, ONE fused custom op)
    p = Exp(2^40 * r), S = rowsum                    (ACT, single pass)
    out = p * (1/S)                                  (DVE tensor_scalar, bf16)

Key tricks:
  - Custom DVE uop (GAT_MASK_LRELU_ANT): z-build + adjacency mask + leaky
    relu + row-max shift fused into one 5-stage vector instruction:
        out = max(0.2*min(in0+s0, in1), min(in0+s0, in1)) - s1
  - min(C*z, adj): |C*z| <= ~4e-11 is far below the smallest positive fp8e5
    value (2^-16), so min selects C*z where adj > 0 and adj (<= 0) where
    masked; the exp then underflows those to ~0 (matching the -1e12 mask).
    Masking before leaky-relu is valid: both commute with min (monotone).
  - EXACT softmax row max at zero bulk cost: max_j e_ij = lrelu(s1_i +
    max_j s2_j) by monotonicity, so the shift is per-partition scalar math.
    Subtracting it BEFORE the bf16 rounding of the score tile kills the
    bf16 absolute error exactly where softmax weight concentrates.
  - Only ONE scalar-engine pass over the matrix (Exp with rowsum accum).
  - adj travels fp8e5 (sign-exact above 2^-16), out travels bf16; both are
    host-side dtype casts only. HBM per core: 8 MB adj + 16 MB out + 2.25 MB h.
"""
import os
import sys

for _p in (
    "/opt/trn_rl_repo",
    "/root/.axon_site/_ro/trn_rl_repo",
):
    if os.path.isdir(_p) and _p not in sys.path:
        sys.path.append(_p)

import numpy as np
import ml_dtypes


def _install_profile_shim():
    """bass_utils' trace path imports antenv.axon_hooks, which this image
    lacks. Provide it (with the ctypes hook into libaxon if available) so a
    BASS_TRACE=1 run profiles instead of crashing. No-op on any failure."""
    import contextlib
    import ctypes
    import types

    if "antenv.axon_hooks" in sys.modules:
        return
    try:
        import antenv
    except ImportError:
        return

    def _make_hook(so_path):
        try:
            lib = ctypes.CDLL(so_path)
        except OSError:
            return None
        if not hasattr(lib, "axon_start_nrt_profile"):
            return None
        lib.axon_start_nrt_profile.argtypes = [
            ctypes.POINTER(ctypes.c_int64),
            ctypes.c_size_t,
        ]
        lib.axon_start_nrt_profile.restype = ctypes.c_int64
        lib.axon_stop_nrt_profile.argtypes = [ctypes.c_char_p]
        lib.axon_stop_nrt_profile.restype = ctypes.c_int64

        @contextlib.contextmanager
        def _hook(output_dir, device_ids):
            import jax

            jax.devices()
            if device_ids:
                ids = (ctypes.c_int64 * len(device_ids))(*device_ids)
                rc = lib.axon_start_nrt_profile(ids, len(device_ids))
            else:
                rc = lib.axon_start_nrt_profile(None, 0)
            if rc != 0:
                raise RuntimeError(f"axon_start_nrt_profile rc={rc}")
            try:
                yield
            finally:
                n = lib.axon_stop_nrt_profile(str(output_dir).encode())
                print(f"profile: {n} file(s) -> {output_dir}", file=sys.stderr)

        return _hook

    hook = [_make_hook("/opt/axon/libaxon_pjrt.so")]
    mod = types.ModuleType("antenv.axon_hooks")
    mod.set_axon_ntff_profile_hook = lambda h: hook.__setitem__(0, h)
    mod.get_axon_ntff_profile_hook = lambda: hook[0]
    sys.modules["antenv.axon_hooks"] = mod
    antenv.axon_hooks = mod


try:
    _install_profile_shim()
except Exception:
    pass

import concourse.bacc as bacc
import concourse.tile as tile
import concourse.bass as bass
from concourse import mybir
from concourse.bass_utils import run_bass_kernel_spmd

N, F_IN, F_OUT = 8192, 128, 64
NCORES = 8
R = N // NCORES          # rows per core (1024)
P = 128                  # SBUF partitions
RT = R // P              # row tiles per core (8)
C = 2.0 ** -40           # exact scale-down of scores
CI = 2.0 ** 40
ALPHA = 0.2              # leaky relu negative slope
F32 = mybir.dt.float32
BF16 = mybir.dt.bfloat16
FP8 = mybir.dt.float8e5
AF = mybir.ActivationFunctionType
ALU = mybir.AluOpType

_CACHED_NC = None
LAST_RESULT = None       # BassKernelResults of the most recent run (for tests)


def _register_gat_op():
    """Register the fused mask+lrelu custom DVE op (idempotent).

    out = max(imm2*min(in0+s0, in1), min(in0+s0, in1)) - s1
    """
    import concourse.dve_ops as dve_ops
    from concourse.dve_spec import Spec, Src0, Src1, C0, C1, C2, maxx, minn, lower
    from concourse.dve_uop import DveOpSpec

    name = "GAT_MASK_LRELU_ANT"
    for op in dve_ops.OPS:
        if op.name == name:
            return op

    def _ref(in0, in1, s0, s1, imm2):
        zb = in0.astype(np.float32) + s0
        m = np.minimum(zb, in1.astype(np.float32))
        u = np.maximum(m * imm2, m)
        return (u - s1).astype(np.float32)

    _zb = Src0 + C0
    _m = minn(_zb, Src1)
    spec = Spec(body=maxx(_m * C2, _m) - C1, reference=_ref)
    row = 1 + len(dve_ops.OPS)
    shas = {}
    for ver in ("v3", "v4"):
        s = DveOpSpec(name=name, opcode=row, uops=lower(spec, ver=ver), rd1_en=True)
        shas[ver] = s.sha(ver)
    op = dve_ops.DveOp(name, spec, subdim=False, uops_sha=shas)
    dve_ops.OPS.append(op)
    dve_ops._SUB_OPCODE_FOR_NAME[name] = row
    dve_ops.CUSTOM_DVE_SPECS[name] = op.spec
    return op


GAT_OP = _register_gat_op()


def build_nc():
    nc = bacc.Bacc("TRN2", target_bir_lowering=False)
    hT_d = nc.dram_tensor("hT", [F_IN, N], BF16, kind="ExternalInput")
    hsT_d = nc.dram_tensor("hsT", [F_IN, R], BF16, kind="ExternalInput")
    adj_d = nc.dram_tensor("adj", [R, N], FP8, kind="ExternalInput")
    wT_d = nc.dram_tensor("wT", [F_OUT, F_IN], F32, kind="ExternalInput")
    a_d = nc.dram_tensor("a", [2 * F_OUT, 1], F32, kind="ExternalInput")
    out_d = nc.dram_tensor("out", [R, N], BF16, kind="ExternalOutput")

    with tile.TileContext(nc) as tc:
        with (
            tc.tile_pool(name="persist", bufs=1) as persist,
            tc.tile_pool(name="hTp", bufs=8) as hTp,
            tc.tile_pool(name="psB", bufs=4, space="PSUM") as psB,
            tc.tile_pool(name="psS", bufs=1, space="PSUM") as psS,
            tc.tile_pool(name="adjp", bufs=3) as adjp,
            tc.tile_pool(name="ep", bufs=6) as ep,
            tc.tile_pool(name="small", bufs=6) as small,
        ):
            # --------- setup: s1 (per-row bias) and BC2 (C*s2 broadcast) ------
            # wT/a2 first (tiny, unblock the wa12 matmul), then hT split
            # across both rings (it is the long pole for BC2).
            wT_sb = persist.tile([F_OUT, F_IN], F32)
            nc.scalar.dma_start(out=wT_sb, in_=wT_d[:, :])
            # a2[o, j] = a[j*64 + o]: a_src / a_dst as two columns
            a2 = persist.tile([F_OUT, 2], F32)
            a_t = a_d.tensor if hasattr(a_d, "tensor") else a_d
            nc.scalar.dma_start(
                out=a2, in_=bass.AP(tensor=a_t, offset=0, ap=[[1, F_OUT], [F_OUT, 2]])
            )
            hsT_sb = persist.tile([P, R], BF16)
            nc.sync.dma_start(out=hsT_sb, in_=hsT_d[:, :])
            hTs = []
            for g in range(8):
                hTc = hTp.tile([P, N // 8], BF16, tag="hTc")
                eng = nc.scalar if g % 2 == 0 else nc.sync
                eng.dma_start(
                    out=hTc, in_=hT_d[:, g * (N // 8):(g + 1) * (N // 8)]
                )
                hTs.append(hTc)

            # wa12[:, j] = w @ (a_src if j==0 else a_dst), one K=64 matmul
            ps_wa = psS.tile([P, 2], F32, tag="pswa")
            nc.tensor.matmul(ps_wa, lhsT=wT_sb, rhs=a2, start=True, stop=True)
            wa12 = persist.tile([P, 2], F32)
            nc.scalar.copy(wa12, ps_wa)

            # W2B[f, p] = C * wa2[f]  (stationary matrix for the BC2 matmuls)
            ones = persist.tile([P, P], BF16)
            nc.vector.memset(ones, 1.0)
            w2b = persist.tile([P, P], BF16)
            nc.vector.tensor_scalar(
                out=w2b, in0=ones, scalar1=wa12[:, 1:2], scalar2=C,
                op0=ALU.mult, op1=ALU.mult,
            )
            wa1c = persist.tile([P, 1], BF16)
            nc.vector.tensor_scalar(
                out=wa1c, in0=wa12[:, 0:1], scalar1=C, scalar2=None, op0=ALU.mult
            )

            # s1c[r, t] = C * s1[t*128 + r]  for this core's 8 row tiles
            ps_s1 = psS.tile([P, RT], F32)
            for t in range(RT):
                nc.tensor.matmul(
                    ps_s1[:, t:t + 1], lhsT=hsT_sb[:, t * P:(t + 1) * P],
                    rhs=wa1c, start=True, stop=True,
                )
            s1c = persist.tile([P, RT], F32)
            nc.scalar.copy(s1c, ps_s1)

            # BC2[p, j] = C * s2[j] for all p  (16 chunks of 512 columns).
            # Copies alternate ACT/DVE; DVE also max-reduces each PSUM chunk
            # so the row max is ready right after the last chunk.
            bc2 = persist.tile([P, N], F32)
            smax16 = persist.tile([P, 16], F32)
            for cg in range(16):
                psb = psB.tile([P, 512], F32, tag="psb")
                nc.tensor.matmul(
                    psb, lhsT=w2b,
                    rhs=hTs[cg // 2][:, (cg % 2) * 512:(cg % 2) * 512 + 512],
                    start=True, stop=True,
                )
                nc.scalar.copy(bc2[:, cg * 512:(cg + 1) * 512], psb)
                nc.vector.tensor_reduce(
                    smax16[:, cg:cg + 1], psb, axis=mybir.AxisListType.X,
                    op=ALU.max,
                )

            # Exact per-row softmax shift: max_j e_ij = lrelu(s1_i + max_j s2_j)
            # (lrelu is monotone). All in the C-scaled domain, per-partition.
            s2maxc = persist.tile([P, 1], F32)
            nc.vector.tensor_reduce(
                s2maxc, smax16, axis=mybir.AxisListType.X, op=ALU.max
            )
            xm = persist.tile([P, RT], F32)
            nc.vector.tensor_scalar(
                out=xm, in0=s1c, scalar1=s2maxc[:, 0:1], scalar2=None, op0=ALU.add
            )
            xm2 = persist.tile([P, RT], F32)
            nc.vector.tensor_scalar(
                out=xm2, in0=xm, scalar1=ALPHA, scalar2=None, op0=ALU.mult
            )
            mc = persist.tile([P, RT], F32)
            nc.vector.tensor_tensor(out=mc, in0=xm, in1=xm2, op=ALU.max)

            # adj loads: SP HWDGE ring (stores ride the ACT ring), fp8 tiles.
            adjts = []
            for t in range(RT):
                adjt = adjp.tile([P, N], FP8, tag="adjt")
                nc.sync.dma_start(
                    out=adjt, in_=adj_d[t * P:(t + 1) * P, :]
                )
                adjts.append(adjt)

            # ---------------- main loop over row tiles (sw-pipelined) ---------
            # chain per tile:
            #   r = lrelu(min(bc2 + s1, adj)) - Mc   (DVE, one fused custom op)
            #   p = Exp(2^40*r), S = rowsum          (ACT, two halves)
            #   out = p * (1/S)                      (DVE)   then store
            # The DVE work for tile t+1 is emitted before tile t's ACT/scale so
            # DVE and ACT overlap across tiles.
            H = N // 2

            def emit_fused(t):
                et = ep.tile([P, N], BF16, tag="et")
                for hx in range(2):
                    sl = slice(hx * H, (hx + 1) * H)
                    nc.vector._custom_dve(
                        GAT_OP, out=et[:, sl], in0=bc2[:, sl],
                        in1=adjts[t][:, sl],
                        s0=s1c[:, t:t + 1], s1=mc[:, t:t + 1], imm2=ALPHA,
                    )
                return et

            LOOKAHEAD = 2
            ets = {t: emit_fused(t) for t in range(min(LOOKAHEAD, RT))}
            for t in range(RT):
                if t + LOOKAHEAD < RT:
                    ets[t + LOOKAHEAD] = emit_fused(t + LOOKAHEAD)
                et = ets.pop(t)
                S2 = small.tile([P, 2], F32, tag="S2")
                for hx in range(2):
                    sl = slice(hx * H, (hx + 1) * H)
                    # p = exp(2^40*r) in place (arg <= 0), S2 half = rowsum
                    nc.scalar.activation(
                        out=et[:, sl], in_=et[:, sl], func=AF.Exp,
                        bias=0.0, scale=CI,
                        accum_out=S2[:, hx:hx + 1],
                    )
                S = small.tile([P, 1], F32, tag="S")
                nc.vector.tensor_scalar(
                    out=S, in0=S2[:, 0:1], scalar1=S2[:, 1:2], scalar2=None,
                    op0=ALU.add,
                )
                rs = small.tile([P, 1], F32, tag="rs")
                nc.vector.reciprocal(rs, S)
                for hx in range(2):
                    sl = slice(hx * H, (hx + 1) * H)
                    nc.vector.tensor_scalar(
                        out=et[:, sl], in0=et[:, sl], scalar1=rs[:, 0:1],
                        scalar2=None, op0=ALU.mult,
                    )
                    # stores issue from idle engines (sync/gpsimd), spreading
                    # across two DMA queues; a backed-up store ring then never
                    # stalls the ACT queue
                    eng = nc.sync if (t + hx) % 2 == 0 else nc.gpsimd
                    eng.dma_start(
                        out=out_d[t * P:(t + 1) * P, sl], in_=et[:, sl]
                    )

    nc.compile()
    return nc


def kernel(h, adj, w, a):
    global _CACHED_NC, LAST_RESULT
    h = np.ascontiguousarray(h, dtype=np.float32)
    adj = np.ascontiguousarray(adj, dtype=np.float32)
    w = np.ascontiguousarray(w, dtype=np.float32)
    a = np.ascontiguousarray(a, dtype=np.float32)

    if _CACHED_NC is None:
        _CACHED_NC = build_nc()
    nc = _CACHED_NC

    hT = np.ascontiguousarray(h.T.astype(ml_dtypes.bfloat16))
    wT = np.ascontiguousarray(w.T)
    in_maps = [
        {
            "hT": hT,
            "hsT": np.ascontiguousarray(hT[:, i * R:(i + 1) * R]),
            "adj": np.ascontiguousarray(
                adj[i * R:(i + 1) * R].astype(ml_dtypes.float8_e5m2)
            ),
            "wT": wT,
            "a": a,
        }
        for i in range(NCORES)
    ]
    res = run_bass_kernel_spmd(nc, in_maps, core_ids=list(range(NCORES)))
    LAST_RESULT = res
    return np.concatenate(
        [r["out"].astype(np.float32) for r in res.results], axis=0
    )


# revision 19
# speedup vs baseline: 1.0294x; 1.0294x over previous
"""GAT attention layer (EEGGraphAttentionLayer) for Trainium2, 8 NeuronCores.

reference math:
    Wh = h @ w                         # (8192, 64)
    e  = leaky_relu((Wh@a_src) + (Wh@a_dst).T, slope=0.2)   # (8192, 8192)
    att = where(adj > 0, e, -1e12)
    out = softmax(att, axis=1)

Sharding: rows of adj/out across 8 cores (1024 rows each); row softmax is
core-local. Each core recomputes the column-score vector s2 = h @ (w@a_dst)
(an N-vector) from the full h instead of communicating. h and w are passed
host-transposed and h/adj precision-reduced (bf16 / fp8e5 -- pure dtype
compression, no host arithmetic); the output is stored bf16 and upcast to
fp32 on the host (exact upcast).

Per-core device pipeline (row tile = [128, 8192] bf16, C = 2^-40 exact):
    wa12 = wT.T @ [a_src|a_dst]                      (PE)
    s1c[:, t] = C * (hsT_tile.T @ wa1)               (PE)   per-row bias
    BC2[p, j] = C * s2[j] via (C*wa2 bcast).T @ hT   (PE)   16 x 512 chunks
    s2maxc    = rowmax(BC2)                          (DVE reduce, one-time)
    Mc[:, t]  = lrelu(s1c + s2maxc)  == C * rowmax(e)  (exact: lrelu monotone)
    r = lrelu(min(BC2 + s1c, adj)) - Mc              (DVE, ONE fused custom op)
    p = Exp(2^40 * r), S = rowsum                    (ACT, single pass)
    out = p * (1/S)                                  (DVE tensor_scalar, bf16)

Key tricks:
  - Custom DVE uop (GAT_MASK_LRELU_ANT): z-build + adjacency mask + leaky
    relu + row-max shift fused into one 5-stage vector instruction:
        out = max(0.2*min(in0+s0, in1), min(in0+s0, in1)) - s1
  - min(C*z, adj): |C*z| <= ~4e-11 is far below the smallest positive fp8e5
    value (2^-16), so min selects C*z where adj > 0 and adj (<= 0) where
    masked; the exp then underflows those to ~0 (matching the -1e12 mask).
    Masking before leaky-relu is valid: both commute with min (monotone).
  - EXACT softmax row max at zero bulk cost: max_j e_ij = lrelu(s1_i +
    max_j s2_j) by monotonicity, so the shift is per-partition scalar math.
    Subtracting it BEFORE the bf16 rounding of the score tile kills the
    bf16 absolute error exactly where softmax weight concentrates.
  - Only ONE scalar-engine pass over the matrix (Exp with rowsum accum).
  - adj travels fp8e5 (sign-exact above 2^-16), out travels bf16; both are
    host-side dtype casts only. HBM per core: 8 MB adj + 16 MB out + 2.25 MB h.
"""
import os
import sys

for _p in (
    "/opt/trn_rl_repo",
    "/root/.axon_site/_ro/trn_rl_repo",
):
    if os.path.isdir(_p) and _p not in sys.path:
        sys.path.append(_p)

import numpy as np
import ml_dtypes


def _install_profile_shim():
    """bass_utils' trace path imports antenv.axon_hooks, which this image
    lacks. Provide it (with the ctypes hook into libaxon if available) so a
    BASS_TRACE=1 run profiles instead of crashing. No-op on any failure."""
    import contextlib
    import ctypes
    import types

    if "antenv.axon_hooks" in sys.modules:
        return
    try:
        import antenv
    except ImportError:
        return

    def _make_hook(so_path):
        try:
            lib = ctypes.CDLL(so_path)
        except OSError:
            return None
        if not hasattr(lib, "axon_start_nrt_profile"):
            return None
        lib.axon_start_nrt_profile.argtypes = [
            ctypes.POINTER(ctypes.c_int64),
            ctypes.c_size_t,
        ]
        lib.axon_start_nrt_profile.restype = ctypes.c_int64
        lib.axon_stop_nrt_profile.argtypes = [ctypes.c_char_p]
        lib.axon_stop_nrt_profile.restype = ctypes.c_int64

        @contextlib.contextmanager
        def _hook(output_dir, device_ids):
            import jax

            jax.devices()
            if device_ids:
                ids = (ctypes.c_int64 * len(device_ids))(*device_ids)
                rc = lib.axon_start_nrt_profile(ids, len(device_ids))
            else:
                rc = lib.axon_start_nrt_profile(None, 0)
            if rc != 0:
                raise RuntimeError(f"axon_start_nrt_profile rc={rc}")
            try:
                yield
            finally:
                n = lib.axon_stop_nrt_profile(str(output_dir).encode())
                print(f"profile: {n} file(s) -> {output_dir}", file=sys.stderr)

        return _hook

    hook = [_make_hook("/opt/axon/libaxon_pjrt.so")]
    mod = types.ModuleType("antenv.axon_hooks")
    mod.set_axon_ntff_profile_hook = lambda h: hook.__setitem__(0, h)
    mod.get_axon_ntff_profile_hook = lambda: hook[0]
    sys.modules["antenv.axon_hooks"] = mod
    antenv.axon_hooks = mod


try:
    _install_profile_shim()
except Exception:
    pass

import concourse.bacc as bacc
import concourse.tile as tile
import concourse.bass as bass
from concourse import mybir
from concourse.bass_utils import run_bass_kernel_spmd

N, F_IN, F_OUT = 8192, 128, 64
NCORES = 8
R = N // NCORES          # rows per core (1024)
P = 128                  # SBUF partitions
RT = R // P              # row tiles per core (8)
C = 2.0 ** -40           # exact scale-down of scores
CI = 2.0 ** 40
ALPHA = 0.2              # leaky relu negative slope
F32 = mybir.dt.float32
BF16 = mybir.dt.bfloat16
FP8 = mybir.dt.float8e5
AF = mybir.ActivationFunctionType
ALU = mybir.AluOpType

_CACHED_NC = None
LAST_RESULT = None       # BassKernelResults of the most recent run (for tests)


def _register_gat_op():
    """Register the fused mask+lrelu custom DVE op (idempotent).

    out = max(imm2*min(in0+s0, in1), min(in0+s0, in1)) - s1
    """
    import concourse.dve_ops as dve_ops
    from concourse.dve_spec import Spec, Src0, Src1, C0, C1, C2, maxx, minn, lower
    from concourse.dve_uop import DveOpSpec

    name = "GAT_MASK_LRELU_ANT"
    for op in dve_ops.OPS:
        if op.name == name:
            return op

    def _ref(in0, in1, s0, s1, imm2):
        zb = in0.astype(np.float32) + s0
        m = np.minimum(zb, in1.astype(np.float32))
        u = np.maximum(m * imm2, m)
        return (u - s1).astype(np.float32)

    _zb = Src0 + C0
    _m = minn(_zb, Src1)
    spec = Spec(body=maxx(_m * C2, _m) - C1, reference=_ref)
    row = 1 + len(dve_ops.OPS)
    shas = {}
    for ver in ("v3", "v4"):
        s = DveOpSpec(name=name, opcode=row, uops=lower(spec, ver=ver), rd1_en=True)
        shas[ver] = s.sha(ver)
    op = dve_ops.DveOp(name, spec, subdim=False, uops_sha=shas)
    dve_ops.OPS.append(op)
    dve_ops._SUB_OPCODE_FOR_NAME[name] = row
    dve_ops.CUSTOM_DVE_SPECS[name] = op.spec
    return op


GAT_OP = _register_gat_op()


def build_nc():
    nc = bacc.Bacc("TRN2", target_bir_lowering=False)
    hT_d = nc.dram_tensor("hT", [F_IN, N], BF16, kind="ExternalInput")
    hsT_d = nc.dram_tensor("hsT", [F_IN, R], BF16, kind="ExternalInput")
    adj_d = nc.dram_tensor("adj", [R, N], FP8, kind="ExternalInput")
    wT_d = nc.dram_tensor("wT", [F_OUT, F_IN], F32, kind="ExternalInput")
    a_d = nc.dram_tensor("a", [2 * F_OUT, 1], F32, kind="ExternalInput")
    out_d = nc.dram_tensor("out", [R, N], BF16, kind="ExternalOutput")

    with tile.TileContext(nc) as tc:
        with (
            tc.tile_pool(name="persist", bufs=1) as persist,
            tc.tile_pool(name="hTp", bufs=8) as hTp,
            tc.tile_pool(name="psB", bufs=4, space="PSUM") as psB,
            tc.tile_pool(name="psS", bufs=1, space="PSUM") as psS,
            tc.tile_pool(name="adjp", bufs=5) as adjp,
            tc.tile_pool(name="ep", bufs=5) as ep,
            tc.tile_pool(name="small", bufs=6) as small,
        ):
            # --------- setup: s1 (per-row bias) and BC2 (C*s2 broadcast) ------
            # wT/a2 first (tiny, unblock the wa12 matmul), then hT split
            # across both rings (it is the long pole for BC2).
            wT_sb = persist.tile([F_OUT, F_IN], F32)
            nc.scalar.dma_start(out=wT_sb, in_=wT_d[:, :])
            # a2[o, j] = a[j*64 + o]: a_src / a_dst as two columns
            a2 = persist.tile([F_OUT, 2], F32)
            a_t = a_d.tensor if hasattr(a_d, "tensor") else a_d
            nc.scalar.dma_start(
                out=a2, in_=bass.AP(tensor=a_t, offset=0, ap=[[1, F_OUT], [F_OUT, 2]])
            )
            hsT_sb = persist.tile([P, R], BF16)
            nc.sync.dma_start(out=hsT_sb, in_=hsT_d[:, :])
            hTs = []
            for g in range(8):
                hTc = hTp.tile([P, N // 8], BF16, tag="hTc")
                eng = nc.scalar if g % 2 == 0 else nc.sync
                eng.dma_start(
                    out=hTc, in_=hT_d[:, g * (N // 8):(g + 1) * (N // 8)]
                )
                hTs.append(hTc)

            # wa12[:, j] = w @ (a_src if j==0 else a_dst), one K=64 matmul
            ps_wa = psS.tile([P, 2], F32, tag="pswa")
            nc.tensor.matmul(ps_wa, lhsT=wT_sb, rhs=a2, start=True, stop=True)
            wa12 = persist.tile([P, 2], F32)
            nc.scalar.copy(wa12, ps_wa)

            # W2B[f, p] = C * wa2[f]  (stationary matrix for the BC2 matmuls)
            ones = persist.tile([P, P], BF16)
            nc.vector.memset(ones, 1.0)
            w2b = persist.tile([P, P], BF16)
            nc.vector.tensor_scalar(
                out=w2b, in0=ones, scalar1=wa12[:, 1:2], scalar2=C,
                op0=ALU.mult, op1=ALU.mult,
            )
            wa1c = persist.tile([P, 1], BF16)
            nc.vector.tensor_scalar(
                out=wa1c, in0=wa12[:, 0:1], scalar1=C, scalar2=None, op0=ALU.mult
            )

            # s1c[r, t] = C * s1[t*128 + r]  for this core's 8 row tiles
            ps_s1 = psS.tile([P, RT], F32)
            for t in range(RT):
                nc.tensor.matmul(
                    ps_s1[:, t:t + 1], lhsT=hsT_sb[:, t * P:(t + 1) * P],
                    rhs=wa1c, start=True, stop=True,
                )
            s1c = persist.tile([P, RT], F32)
            nc.scalar.copy(s1c, ps_s1)

            # BC2[p, j] = C * s2[j] for all p  (16 chunks of 512 columns).
            # Copies alternate ACT/DVE; DVE also max-reduces each PSUM chunk
            # so the row max is ready right after the last chunk.
            bc2 = persist.tile([P, N], F32)
            smax16 = persist.tile([P, 16], F32)
            for cg in range(16):
                psb = psB.tile([P, 512], F32, tag="psb")
                nc.tensor.matmul(
                    psb, lhsT=w2b,
                    rhs=hTs[cg // 2][:, (cg % 2) * 512:(cg % 2) * 512 + 512],
                    start=True, stop=True,
                )
                nc.scalar.copy(bc2[:, cg * 512:(cg + 1) * 512], psb)
                nc.vector.tensor_reduce(
                    smax16[:, cg:cg + 1], psb, axis=mybir.AxisListType.X,
                    op=ALU.max,
                )

            # Exact per-row softmax shift: max_j e_ij = lrelu(s1_i + max_j s2_j)
            # (lrelu is monotone). All in the C-scaled domain, per-partition.
            s2maxc = persist.tile([P, 1], F32)
            nc.vector.tensor_reduce(
                s2maxc, smax16, axis=mybir.AxisListType.X, op=ALU.max
            )
            xm = persist.tile([P, RT], F32)
            nc.vector.tensor_scalar(
                out=xm, in0=s1c, scalar1=s2maxc[:, 0:1], scalar2=None, op0=ALU.add
            )
            xm2 = persist.tile([P, RT], F32)
            nc.vector.tensor_scalar(
                out=xm2, in0=xm, scalar1=ALPHA, scalar2=None, op0=ALU.mult
            )
            mc = persist.tile([P, RT], F32)
            nc.vector.tensor_tensor(out=mc, in0=xm, in1=xm2, op=ALU.max)

            # adj loads: SP HWDGE ring (stores ride the ACT ring), fp8 tiles.
            adjts = []
            for t in range(RT):
                adjt = adjp.tile([P, N], FP8, tag="adjt")
                nc.sync.dma_start(
                    out=adjt, in_=adj_d[t * P:(t + 1) * P, :]
                )
                adjts.append(adjt)

            # ---------------- main loop over row tiles (sw-pipelined) ---------
            # chain per tile:
            #   r = lrelu(min(bc2 + s1, adj)) - Mc   (DVE, one fused custom op)
            #   p = Exp(2^40*r), S = rowsum          (ACT, two halves)
            #   out = p * (1/S)                      (DVE)   then store
            # The DVE work for tile t+1 is emitted before tile t's ACT/scale so
            # DVE and ACT overlap across tiles.
            H = N // 2

            def emit_fused(t):
                et = ep.tile([P, N], BF16, tag="et")
                nc.vector._custom_dve(
                    GAT_OP, out=et, in0=bc2, in1=adjts[t],
                    s0=s1c[:, t:t + 1], s1=mc[:, t:t + 1], imm2=ALPHA,
                )
                return et

            LOOKAHEAD = 2
            ets = {t: emit_fused(t) for t in range(min(LOOKAHEAD, RT))}
            for t in range(RT):
                if t + LOOKAHEAD < RT:
                    ets[t + LOOKAHEAD] = emit_fused(t + LOOKAHEAD)
                et = ets.pop(t)
                S2 = small.tile([P, 2], F32, tag="S2")
                for hx in range(2):
                    sl = slice(hx * H, (hx + 1) * H)
                    # p = exp(2^40*r) in place (arg <= 0), S2 half = rowsum
                    nc.scalar.activation(
                        out=et[:, sl], in_=et[:, sl], func=AF.Exp,
                        bias=0.0, scale=CI,
                        accum_out=S2[:, hx:hx + 1],
                    )
                S = small.tile([P, 1], F32, tag="S")
                nc.vector.tensor_scalar(
                    out=S, in0=S2[:, 0:1], scalar1=S2[:, 1:2], scalar2=None,
                    op0=ALU.add,
                )
                rs = small.tile([P, 1], F32, tag="rs")
                nc.vector.reciprocal(rs, S)
                for hx in range(2):
                    sl = slice(hx * H, (hx + 1) * H)
                    nc.vector.tensor_scalar(
                        out=et[:, sl], in0=et[:, sl], scalar1=rs[:, 0:1],
                        scalar2=None, op0=ALU.mult,
                    )
                    # stores issue from the (otherwise idle) sync engine: a
                    # backed-up store ring then never stalls the ACT queue.
                    # The last tiles go down the by-then-empty scalar ring
                    # (those issues sit after all exps in the ACT queue).
                    eng = nc.scalar if t >= 6 else nc.sync
                    eng.dma_start(
                        out=out_d[t * P:(t + 1) * P, sl], in_=et[:, sl]
                    )

    nc.compile()
    return nc


def kernel(h, adj, w, a):
    global _CACHED_NC, LAST_RESULT
    h = np.ascontiguousarray(h, dtype=np.float32)
    adj = np.ascontiguousarray(adj, dtype=np.float32)
    w = np.ascontiguousarray(w, dtype=np.float32)
    a = np.ascontiguousarray(a, dtype=np.float32)

    if _CACHED_NC is None:
        _CACHED_NC = build_nc()
    nc = _CACHED_NC

    hT = np.ascontiguousarray(h.T.astype(ml_dtypes.bfloat16))
    wT = np.ascontiguousarray(w.T)
    in_maps = [
        {
            "hT": hT,
            "hsT": np.ascontiguousarray(hT[:, i * R:(i + 1) * R]),
            "adj": np.ascontiguousarray(
                adj[i * R:(i + 1) * R].astype(ml_dtypes.float8_e5m2)
            ),
            "wT": wT,
            "a": a,
        }
        for i in range(NCORES)
    ]
    res = run_bass_kernel_spmd(nc, in_maps, core_ids=list(range(NCORES)))
    LAST_RESULT = res
    return np.concatenate(
        [r["out"].astype(np.float32) for r in res.results], axis=0
    )
